# revision 2
# baseline (speedup 1.0000x reference)
"""DogeCDMoME (product-key MoE routing) Trainium2 kernel — v2.

Sharding: data-parallel over tokens across 8 NeuronCores (256 tokens each).
The two 128-token matmul tiles of a core are processed in LOCKSTEP so every
weight tile is streamed from HBM exactly once per core (the v1 kernel
streamed W_up/W_down/W_q once per 128-token pass, i.e. twice).

Transposed dense formulation: stages A (x@W_up) and C (h@W_q) compute the
TRANSPOSED output directly (W^T @ x with the weight chunk as the stationary
operand), so stage outputs land already in the [feature, token] layout the
next stage consumes as lhsT — no per-block PE transposes of activations.
Only x (16/tile) and h (8/tile) are PE-transposed.

Matmul precision is configurable per dense stage (A, B, C):
  "f32"  — native fp32, 4 cycles/row.
  "f32r" — fp32-reduced PE mode (~11 mantissa bits), 1 cycle/row; operands
           must be produced by an instruction that rounds to f32r.
  "s3"   — 3-pass bf16 split (hi@hi + hi@lo + lo@hi), ~15-16 effective
           mantissa bits at 3 cycles/row.
Stage D (sim = q @ keys^T) stays fp32: it is the most flip-sensitive and
the cheapest. The routing top-k is extremely sensitive to matmul noise
(bf16 flips ~1.2% of expert selections and fails the 2e-2 gate), which is
why plain bf16 is not an option for A/B/C either.

Pipeline per rep (256 tokens):
  X:  x -> x^T (PE transpose), split/round per mode A
  A+B interleaved over 32 s-blocks of 256 cols (dc-major weight streaming;
      W_up strip [128,256] per (dc, blk)): us[s,tok256] accumulates over dc,
      silu on ScalarE+DVE produces at[s,tok] directly in B's lhsT layout;
      W_down row-chunk [128,1024] per s-chunk streams in and hs[tt][tok,P]
      accumulates in PSUM across all 64 s-chunks.
  H:  hs -> h_sb (fp32, for the g-dot) and h^T (mode-C dtype, for stage C).
  C:  q^T[cq,tok256] computed per 128-chunk, staged to SBUF fp32.
  D:  sim[tok,keys] per (plane,head) from q^T chunks vs resident keys^T.
  Tails per 128-token tile: vector top-8 per plane, cartesian 8x8 combine,
      top-8-of-64, expert-id recovery via one-hot dot, softmax; indirect
      row gathers of up_embed/down_embed (one row per partition per call),
      g = <h,ue> and the weighted down_embed accumulation on DVE.
"""

import numpy as np
from contextlib import ExitStack

import concourse.bass as bass
import concourse.mybir as mybir
import concourse.tile as tile
from concourse.bass import IndirectOffsetOnAxis
from concourse.masks import make_identity

AF = mybir.ActivationFunctionType
ALU = mybir.AluOpType
DT = mybir.dt

N_CORES = 8
T_TOTAL = 2048
T_CORE = T_TOTAL // N_CORES      # 256
TT = 128                         # tokens per matmul tile
D = 2048                         # model dim
S = 8192                         # FFN hidden
P = 1024                         # value dim
CQ = 4096                        # W_q output dim = 2*H*(P//2)
H = 4                            # heads
NK = 128                         # keys per plane
TK = 8                           # top-k
E = 16384                        # experts

F32 = DT.float32
F32R = DT.float32r
BF16 = DT.bfloat16

# per-stage matmul mode: A (x@W_up), B (a@W_down), C (h@W_q)
MODES = ("f32", "f32", "f32")
WUP_3D = True  # kernel feeds W_up reshaped to [16,128,8192]

_WAIT_EXEMPT = {"InstEventSemaphore"}


def _legalize_waits(nc, keep=1):
    """This walrus build rejects >1 attached sync wait per instruction and
    the EVENT_SEMAPHORE_RANGE_CLEAR encoding; hoist extra waits onto
    standalone EventSemaphore instructions and expand range-clears."""
    import re

    n_fix = 0
    for f in nc.m.functions:
        for bb in f.blocks:
            il = bb.instructions
            i = 0
            while i < len(il):
                ins = il[i]
                tname = type(ins).__name__
                if tname == "InstISA" and getattr(ins, "isa_opcode", None) == 176:
                    m = re.search(r"range_first=(\d+) range_last=(\d+)",
                                  ins.concise())
                    lo, hi = int(m.group(1)), int(m.group(2))
                    il.pop(i)
                    del nc.inst_map[ins.name]
                    for k, sem in enumerate(range(lo, hi + 1)):
                        clr = mybir.InstEventSemaphore(
                            name=f"{ins.name}_clr{k}",
                            engine=ins.engine,
                            ins=[],
                            outs=[],
                            sync_info=mybir.SyncInfo(
                                on_wait=list(ins.sync_info.on_wait)
                                if ins.sync_info and k == 0 else [],
                                on_update=[mybir.SyncUpdate(
                                    sync_type="semaphore", id=sem,
                                    ant_name=f"clr{sem}",
                                    update_mode="sem-wr-imm", update_value=0,
                                )],
                            ),
                        )
                        nc.inst_map[clr.name] = clr
                        il.insert(i + k, clr)
                    i += hi - lo + 1
                    continue
                si = ins.sync_info
                waits = list(si.on_wait) if si is not None and si.on_wait else []
                if tname not in _WAIT_EXEMPT and len(waits) > keep:
                    extra, kept = waits[:-keep], waits[-keep:]
                    for k, w in enumerate(extra):
                        nop = mybir.InstEventSemaphore(
                            name=f"{ins.name}_wfix{k}",
                            engine=ins.engine,
                            ins=[],
                            outs=[],
                            sync_info=mybir.SyncInfo(on_wait=[w], on_update=[]),
                        )
                        nc.inst_map[nop.name] = nop
                        il.insert(i, nop)
                        i += 1
                        n_fix += 1
                    ins.sync_info = mybir.SyncInfo(
                        on_wait=kept, on_update=list(si.on_update or [])
                    )
                i += 1
    return n_fix


def build_bass(reps=1, modes=MODES):
    nc = bass.Bass(trn_type="TRN2")

    x_d = nc.dram_tensor("x", [T_CORE, D], F32, kind="ExternalInput")
    # [16,128,S] view of [D,S]: lets one DMA fetch several 128-row chunks
    # of one column block via an AP transpose (fewer, fatter triggers)
    wup_d = nc.dram_tensor("W_up", [D // 128, 128, S], F32, kind="ExternalInput")
    wdn_d = nc.dram_tensor("W_down", [S, P], F32, kind="ExternalInput")
    wq_d = nc.dram_tensor("W_q", [P, CQ], F32, kind="ExternalInput")
    keys_d = nc.dram_tensor("keys", [H, NK, 2, P // 2], F32, kind="ExternalInput")
    ue_d = nc.dram_tensor("up_embed", [E, P], F32, kind="ExternalInput")
    de_d = nc.dram_tensor("down_embed", [E, D], F32, kind="ExternalInput")
    out_d = nc.dram_tensor("out", [T_CORE, D], F32, kind="ExternalOutput")

    with tile.TileContext(nc) as tc, ExitStack() as ctx:
        env = {"x_d": x_d, "wup_d": wup_d, "wdn_d": wdn_d, "wq_d": wq_d,
               "ue_d": ue_d, "de_d": de_d, "out_d": out_d}

        # ---------------- pools ----------------
        def pool(name, bufs=1, space=None):
            kw = {"space": space} if space else {}
            env[name] = ctx.enter_context(tc.tile_pool(name=name, bufs=bufs, **kw))

        mA, mB, mC = modes
        pool("cpool")
        pool("xpool", 2)
        pool("xt2pool")
        # W_up streams as [128, 4*256] batches (4 row-chunks of one column
        # block per DMA). All 16 dc chunks of a block stay live through the
        # block's matmuls: whichever pool the matmuls read from needs >=4
        # batch slots (+prefetch); raw-only-for-prep pools recycle fast.
        pool("wupbat", 5 if mA == "f32" else 3)
        pool("wupuse", 5)
        pool("wdnraw", 4)
        pool("wdnuse", 5)
        pool("wqraw", 9 if mC == "f32" else 3)
        pool("wquse", 9)
        pool("atpool", 4 if mB == "s3" else 6)
        pool("sgpool", 3)
        pool("uspool", 4)
        pool("hpool")
        pool("htpool")
        pool("qupool", 4 if mB == "s3" else 5)
        pool("simpool")
        pool("tkpool", 2)
        pool("gpool", 1)
        pool("uepool", 2)
        pool("depool", 2)
        pool("accpool")

        pool("ps_mm", 2, "PSUM")    # us (A) / qs (C): [128,256]
        pool("ps_hs", 1, "PSUM")    # hs[tt]: [128,1024] x2
        pool("ps_tr", 2, "PSUM")    # transposes + stage-D sim: [128,128]

        # ---------------- constants ----------------
        ident = cpool = env["cpool"]
        ident = env["cpool"].tile([128, 128], F32, tag="ident")
        make_identity(nc, ident[:])
        env["ident"] = ident

        iota_i = env["cpool"].tile([128, 64], DT.int32, tag="iota_i")
        nc.gpsimd.iota(iota_i[:], pattern=[[1, 64]], base=0, channel_multiplier=0)
        iota_f = env["cpool"].tile([128, 64], F32, tag="iota_f")
        nc.vector.tensor_copy(iota_f[:], iota_i[:])
        env["iota_f"] = iota_f

        # keys, transposed: keysT[:, ((p*H+h)*4+dc)*128 : +128] = keys[h,:,p,dc]^T
        keysT = env["cpool"].tile([128, 2 * H * 4 * 128], F32, tag="keysT")
        for p in range(2):
            for hh in range(H):
                for dc in range(4):
                    kst = env["cpool"].tile([128, 128], F32, tag="kstage")
                    nc.scalar.dma_start(
                        kst[:], keys_d[hh, :, p, dc * 128:(dc + 1) * 128]
                    )
                    ptr = env["ps_tr"].tile([128, 128], F32, tag="tr")
                    nc.tensor.transpose(ptr[:], kst[:], ident[:])
                    col = ((p * H + hh) * 4 + dc) * 128
                    nc.vector.tensor_copy(keysT[:, col:col + 128], ptr[:])
        env["keysT"] = keysT

        for _rep in range(reps):
            _pipeline(nc, tc, env, modes)

    _legalize_waits(nc)
    return nc


def _stage_dt(mode):
    return {"f32": F32, "f32r": F32R, "s3": BF16}[mode]


def _mm_passes(mode):
    """[(lhs_variant, rhs_variant)] per mode; variants index (main, lo)."""
    if mode == "s3":
        return [(0, 0), (0, 1), (1, 0)]
    return [(0, 0)]


def _emit_b(nc, item, hs, mB):
    sc, at, wdn = item
    passes_b = _mm_passes(mB)
    for tt in range(2):
        for pi, (lv, rv) in enumerate(passes_b):
            for half in range(2):
                nc.tensor.matmul(
                    hs[tt][:, half * 512:(half + 1) * 512],
                    lhsT=at[lv][:, tt * 128:(tt + 1) * 128],
                    rhs=wdn[rv][:, half * 512:(half + 1) * 512],
                    start=(sc == 0 and pi == 0),
                    stop=(sc == 63 and pi == len(passes_b) - 1),
                )


def _pipeline(nc, tc, env, modes):
    mA, mB, mC = modes
    (x_d, wup_d, wdn_d, wq_d, ue_d, de_d, out_d) = (
        env["x_d"], env["wup_d"], env["wdn_d"], env["wq_d"],
        env["ue_d"], env["de_d"], env["out_d"])
    ident, iota_f, keysT = env["ident"], env["iota_f"], env["keysT"]
    ps_mm, ps_hs, ps_tr = env["ps_mm"], env["ps_hs"], env["ps_tr"]

    # ---- X: load + transpose into xt2 (both tiles interleaved 256-wise) ----
    # xt2[:, dc*256 + tt*128 + t] = x[tt*128 + t, dc*128 + p]
    if mA == "s3":
        xt2_hi = env["xt2pool"].tile([128, 16 * 256], BF16, tag="xt2_hi")
        xt2_lo = env["xt2pool"].tile([128, 16 * 256], BF16, tag="xt2_lo")
        xt2 = (xt2_hi, xt2_lo)
    else:
        xt2_m = env["xt2pool"].tile([128, 16 * 256], _stage_dt(mA), tag="xt2")
        xt2 = (xt2_m, None)
    for tt in range(2):
        for xh in range(2):
            x_sb = env["xpool"].tile([128, D // 2], F32, tag="x_sb")
            nc.scalar.dma_start(
                x_sb[:], x_d[tt * TT:(tt + 1) * TT,
                             xh * (D // 2):(xh + 1) * (D // 2)])
            for dk in range(8):
                dc = xh * 8 + dk
                ptr = ps_tr.tile([128, 128], F32, tag="tr")
                nc.tensor.transpose(ptr[:], x_sb[:, dk * 128:(dk + 1) * 128],
                                    ident[:])
                col = dc * 256 + tt * 128
                if mA == "s3":
                    nc.scalar.copy(xt2_hi[:, col:col + 128], ptr[:])
                    nc.vector.tensor_tensor(
                        out=xt2_lo[:, col:col + 128], in0=ptr[:],
                        in1=xt2_hi[:, col:col + 128], op=ALU.subtract)
                else:
                    nc.scalar.copy(xt2[0][:, col:col + 128], ptr[:])

    # ---- A + B interleaved over 32 blocks of 256 S-columns (2 s-chunks) ---
    hs = [ps_hs.tile([128, P], F32, tag=f"hs{tt}", name=f"hs{tt}")
          for tt in range(2)]
    b_pend = []
    for blk in range(32):
        c0 = blk * 256
        # W_up batches: [128, 4x256] = 4 row-chunks of this column block in
        # one DMA (AP-transposed [4,128,256] -> [128,4,256] iteration)
        strips = []
        for b in range(4):
            bat = env["wupbat"].tile([128, 1024], F32, tag="wupbat")
            nc.sync.dma_start(
                bat[:],
                wup_d[4 * b:4 * b + 4, :, c0:c0 + 256].transpose([1, 0, 2]))
            if mA == "s3":
                hi = env["wupuse"].tile([128, 1024], BF16, tag="wuphi")
                lo = env["wupuse"].tile([128, 1024], BF16, tag="wuplo")
                nc.gpsimd.tensor_copy(hi[:], bat[:])
                nc.gpsimd.tensor_tensor(out=lo[:], in0=bat[:], in1=hi[:],
                                        op=ALU.subtract)
                for i in range(4):
                    strips.append((hi[:, i * 256:(i + 1) * 256],
                                   lo[:, i * 256:(i + 1) * 256]))
            elif mA == "f32r":
                use = env["wupuse"].tile([128, 1024], F32R, tag="wupuse")
                nc.gpsimd.tensor_copy(use[:], bat[:])
                for i in range(4):
                    strips.append((use[:, i * 256:(i + 1) * 256], None))
            else:
                for i in range(4):
                    strips.append((bat[:, i * 256:(i + 1) * 256], None))
        passes_a = _mm_passes(mA)
        # both s-chunks' accumulators open at once, iterating batch-major so
        # each W_up batch is fully consumed (and freed) before the next —
        # turns the batch pool into real prefetch depth
        uss = [ps_mm.tile([128, 256], F32, tag="mm", name="us") for _ in range(2)]
        n_acc = len(passes_a) * 16
        ks = [0, 0]
        for (lv, rv) in passes_a:
            for dc in range(16):
                for j in range(2):
                    nc.tensor.matmul(
                        uss[j][:],
                        lhsT=strips[dc][lv][:, j * 128:(j + 1) * 128],
                        rhs=xt2[rv][:, dc * 256:(dc + 1) * 256],
                        start=(ks[j] == 0), stop=(ks[j] == n_acc - 1),
                    )
                    ks[j] += 1
        for j in range(2):
            sc = blk * 2 + j
            # drain PSUM via ScalarE right away (tail-free engine) so the
            # accumulator slot frees without waiting on the DVE silu chain
            us = env["uspool"].tile([128, 256], F32, tag="us_sb")
            nc.scalar.copy(us[:], uss[j][:])
            # silu: at = us * sigmoid(us), directly in B's lhsT layout
            sg = env["sgpool"].tile([128, 256], F32, tag="sg")
            nc.scalar.activation(sg[:], us[:], AF.Sigmoid)
            if mB == "s3":
                at_f = env["atpool"].tile([128, 256], F32, tag="at_f")
                nc.vector.tensor_tensor(out=at_f[:], in0=sg[:], in1=us[:],
                                        op=ALU.mult)
                at_hi = env["atpool"].tile([128, 256], BF16, tag="at_hi")
                at_lo = env["atpool"].tile([128, 256], BF16, tag="at_lo")
                nc.vector.tensor_copy(at_hi[:], at_f[:])
                nc.vector.tensor_tensor(out=at_lo[:], in0=at_f[:],
                                        in1=at_hi[:], op=ALU.subtract)
                at = (at_hi, at_lo)
            else:
                at_m = env["atpool"].tile([128, 256], _stage_dt(mB), tag="at")
                nc.vector.tensor_tensor(out=at_m[:], in0=sg[:], in1=us[:],
                                        op=ALU.mult)
                at = (at_m, None)

            # B inputs: stream W_down row-chunk sc
            wdn_raw = env["wdnraw"].tile([128, P], F32, tag="wdnraw")
            nc.scalar.dma_start(wdn_raw[:], wdn_d[sc * 128:(sc + 1) * 128, :])
            if mB == "s3":
                w_hi = env["wdnuse"].tile([128, P], BF16, tag="wdnhi")
                w_lo = env["wdnuse"].tile([128, P], BF16, tag="wdnlo")
                nc.scalar.copy(w_hi[:], wdn_raw[:])
                nc.gpsimd.tensor_tensor(out=w_lo[:], in0=wdn_raw[:],
                                        in1=w_hi[:], op=ALU.subtract)
                wdn = (w_hi, w_lo)
            elif mB == "f32r":
                w_use = env["wdnuse"].tile([128, P], F32R, tag="wdnuse")
                nc.scalar.copy(w_use[:], wdn_raw[:])
                wdn = (w_use, None)
            else:
                wdn = (wdn_raw, None)
            # software-pipeline B one s-chunk behind A: the silu chain
            # (ScalarE+Pool) for sc produces `at` while PE runs A(sc+1), so
            # B(sc) never stalls the PE stream
            b_pend.append((sc, at, wdn))
            if len(b_pend) > 4:
                _emit_b(nc, b_pend.pop(0), hs, mB)
    while b_pend:
        _emit_b(nc, b_pend.pop(0), hs, mB)

    # ---- H: hs -> h_sb (fp32) and h^T (mode-C dtype) ----
    h_sbs = []
    for tt in range(2):
        h_sb = env["hpool"].tile([128, P], F32, tag=f"h_sb{tt}")
        nc.vector.tensor_copy(h_sb[:], hs[tt][:])
        h_sbs.append(h_sb)
    if mC == "s3":
        ht2_hi = env["htpool"].tile([128, 8 * 256], BF16, tag="ht2_hi")
        ht2_lo = env["htpool"].tile([128, 8 * 256], BF16, tag="ht2_lo")
        ht2 = (ht2_hi, ht2_lo)
    else:
        ht2_m = env["htpool"].tile([128, 8 * 256], _stage_dt(mC), tag="ht2")
        ht2 = (ht2_m, None)
    for tt in range(2):
        for pc in range(8):
            ptr = ps_tr.tile([128, 128], F32, tag="tr")
            nc.tensor.transpose(ptr[:], h_sbs[tt][:, pc * 128:(pc + 1) * 128],
                                ident[:])
            col = pc * 256 + tt * 128
            if mC == "s3":
                nc.scalar.copy(ht2_hi[:, col:col + 128], ptr[:])
                nc.vector.tensor_tensor(
                    out=ht2_lo[:, col:col + 128], in0=ptr[:],
                    in1=ht2_hi[:, col:col + 128], op=ALU.subtract)
            else:
                nc.scalar.copy(ht2[0][:, col:col + 128], ptr[:])

    # ---- C + D + tails: heads-major so each head's tail (DVE/ScalarE/
    # gathers) overlaps the next head's C/D (PE/DMA) ----
    sim_p = [[env["simpool"].tile([128, H * NK], F32, tag=f"sim{tt}_{p}",
                                  name=f"sim{tt}_{p}")
              for p in range(2)] for tt in range(2)]
    tails = [_tail_state(env, tt) for tt in range(2)]
    passes_c = _mm_passes(mC)
    for hp in range(4):
        for p in range(2):
            ph = p * 4 + hp
            # W_q strips for this 512-col block, pc-major
            wq_strips = []
            for pc in range(8):
                raw = env["wqraw"].tile([128, 512], F32, tag="wqraw")
                nc.scalar.dma_start(
                    raw[:],
                    wq_d[pc * 128:(pc + 1) * 128, ph * 512:(ph + 1) * 512])
                if mC == "s3":
                    hi = env["wquse"].tile([128, 512], BF16, tag="wqhi")
                    lo = env["wquse"].tile([128, 512], BF16, tag="wqlo")
                    nc.scalar.copy(hi[:], raw[:])
                    nc.gpsimd.tensor_tensor(out=lo[:], in0=raw[:], in1=hi[:],
                                            op=ALU.subtract)
                    wq_strips.append((hi, lo))
                elif mC == "f32r":
                    use = env["wquse"].tile([128, 512], F32R, tag="wquse")
                    nc.scalar.copy(use[:], raw[:])
                    wq_strips.append((use, None))
                else:
                    wq_strips.append((raw, None))
            q_us = []
            for j in range(4):   # cq 128-chunks within the (p,hh) block
                qs = ps_mm.tile([128, 256], F32, tag="mm")
                n_acc = len(passes_c) * 8
                k = 0
                for (lv, rv) in passes_c:
                    for pc in range(8):
                        nc.tensor.matmul(
                            qs[:],
                            lhsT=wq_strips[pc][lv][:, j * 128:(j + 1) * 128],
                            rhs=ht2[rv][:, pc * 256:(pc + 1) * 256],
                            start=(k == 0), stop=(k == n_acc - 1),
                        )
                        k += 1
                q_u = env["qupool"].tile([128, 256], F32, tag="q_u")
                nc.scalar.copy(q_u[:], qs[:])
                q_us.append(q_u)
            # D: sim for this (p, hh=hp), both tiles, fp32
            for tt in range(2):
                dsim = ps_tr.tile([128, 128], F32, tag="tr")
                for j in range(4):
                    nc.tensor.matmul(
                        dsim[:],
                        lhsT=q_us[j][:, tt * 128:(tt + 1) * 128],
                        rhs=keysT[:, (ph * 4 + j) * 128:
                                  (ph * 4 + j + 1) * 128],
                        start=(j == 0), stop=(j == 3),
                    )
                nc.scalar.copy(
                    sim_p[tt][p][:, hp * NK:(hp + 1) * NK], dsim[:])
        for tt in range(2):
            _tail_head(nc, env, tails[tt], sim_p[tt], h_sbs[tt], iota_f,
                       ue_d, de_d, hp)
    for tt in range(2):
        nc.sync.dma_start(out_d[tt * TT:(tt + 1) * TT, :],
                          tails[tt]["acc"][:])


def _tail_state(env, tt):
    gpool = env["gpool"]
    return {
        "g_all": gpool.tile([128, H * TK], F32, tag=f"g_all{tt}",
                            name="g_all"),
        "gate_all": gpool.tile([128, H * TK], F32, tag=f"gate_all{tt}",
                               name="gate_all"),
        "eidx_f": gpool.tile([128, H * TK], F32, tag=f"eidx_f{tt}",
                             name="eidx_f"),
        "w_all": gpool.tile([128, H * TK], F32, tag=f"w_all{tt}",
                            name="w_all"),
        "ei32": gpool.tile([128, H * TK], DT.int32, tag=f"ei32{tt}",
                           name="ei32"),
        "acc": env["accpool"].tile([128, D], F32, tag=f"acc{tt}", name="acc"),
    }


def _tail_head(nc, env, st, sim_pt, h_sb, iota_f, ue_d, de_d, hh):
    tkpool, gpool = env["tkpool"], env["gpool"]
    g_all, gate_all, eidx_f, w_all, ei32, acc = (
        st["g_all"], st["gate_all"], st["eidx_f"], st["w_all"], st["ei32"],
        st["acc"])
    gscr = gpool.tile([128, P], F32, tag="gscr", bufs=1)

    if True:
        sx = tkpool.tile([128, 8], F32, tag="sx")
        sy = tkpool.tile([128, 8], F32, tag="sy")
        ix = tkpool.tile([128, 8], DT.uint32, tag="ix")
        iy = tkpool.tile([128, 8], DT.uint32, tag="iy")
        simx = sim_pt[0][:, hh * NK:(hh + 1) * NK]
        simy = sim_pt[1][:, hh * NK:(hh + 1) * NK]
        nc.vector.max(sx[:], simx)
        nc.vector.max_index(ix[:], sx[:], simx)
        nc.vector.max(sy[:], simy)
        nc.vector.max_index(iy[:], sy[:], simy)

        ixf = tkpool.tile([128, 8], F32, tag="ixf")
        iyf = tkpool.tile([128, 8], F32, tag="iyf")
        nc.vector.tensor_copy(ixf[:], ix[:])
        nc.vector.tensor_copy(iyf[:], iy[:])
        cix = tkpool.tile([128, 8], F32, tag="cix")
        nc.vector.tensor_scalar_mul(cix[:], ixf[:], float(NK))

        allsc = tkpool.tile([128, 64], F32, tag="allsc")
        allid = tkpool.tile([128, 64], F32, tag="allid")
        for i in range(8):
            nc.vector.tensor_scalar_add(
                allsc[:, i * 8:(i + 1) * 8], sy[:], sx[:, i:i + 1]
            )
            nc.vector.tensor_scalar_add(
                allid[:, i * 8:(i + 1) * 8], iyf[:], cix[:, i:i + 1]
            )

        msc = tkpool.tile([128, 8], F32, tag="msc")
        pos = tkpool.tile([128, 8], DT.uint32, tag="pos")
        nc.vector.max(msc[:], allsc[:])
        nc.vector.max_index(pos[:], msc[:], allsc[:])
        posf = tkpool.tile([128, 8], F32, tag="posf")
        nc.vector.tensor_copy(posf[:], pos[:])

        oh = tkpool.tile([128, 64], F32, tag="oh")
        ohscr = tkpool.tile([128, 64], F32, tag="ohscr")
        for s in range(8):
            nc.vector.tensor_scalar(
                oh[:], iota_f[:], posf[:, s:s + 1], None, op0=ALU.is_equal
            )
            nc.vector.scalar_tensor_tensor(
                out=ohscr[:],
                in0=oh[:],
                scalar=1.0,
                in1=allid[:],
                op0=ALU.bypass,
                op1=ALU.mult,
                accum_out=eidx_f[:, hh * TK + s:hh * TK + s + 1],
            )

        # softmax over the 8 scores
        rmax = tkpool.tile([128, 1], F32, tag="rmax")
        nc.vector.tensor_reduce(
            rmax[:], msc[:], axis=mybir.AxisListType.X, op=ALU.max
        )
        nrmax = tkpool.tile([128, 1], F32, tag="nrmax")
        nc.vector.tensor_scalar_mul(nrmax[:], rmax[:], -1.0)
        esc = tkpool.tile([128, 8], F32, tag="esc")
        ssum = tkpool.tile([128, 1], F32, tag="ssum")
        nc.scalar.activation(
            esc[:], msc[:], AF.Exp, bias=nrmax[:, :], accum_out=ssum[:]
        )
        rinv = tkpool.tile([128, 1], F32, tag="rinv")
        nc.vector.reciprocal(rinv[:], ssum[:])
        nc.vector.tensor_scalar_mul(
            gate_all[:, hh * TK:(hh + 1) * TK], esc[:], rinv[:, :]
        )

        # ---- per-head gather + g + weights + combine ----
        hsl = slice(hh * TK, (hh + 1) * TK)
        nc.vector.tensor_copy(ei32[:, hsl], eidx_f[:, hsl])
        for s in range(hh * TK, (hh + 1) * TK):
            ue_t = env["uepool"].tile([128, P], F32, tag="ue_t", name="ue_t")
            nc.gpsimd.indirect_dma_start(
                out=ue_t[:],
                out_offset=None,
                in_=ue_d[:],
                in_offset=IndirectOffsetOnAxis(ap=ei32[:, s:s + 1], axis=0),
            )
            nc.vector.scalar_tensor_tensor(
                out=gscr[:],
                in0=ue_t[:],
                scalar=1.0,
                in1=h_sb[:],
                op0=ALU.bypass,
                op1=ALU.mult,
                accum_out=g_all[:, s:s + 1],
            )
        gsig = tkpool.tile([128, TK], F32, tag="gsig")
        nc.scalar.activation(gsig[:], g_all[:, hsl], AF.Sigmoid)
        gsil = tkpool.tile([128, TK], F32, tag="gsil")
        nc.vector.tensor_tensor(
            out=gsil[:], in0=gsig[:], in1=g_all[:, hsl], op=ALU.mult
        )
        nc.vector.tensor_tensor(
            out=w_all[:, hsl], in0=gsil[:], in1=gate_all[:, hsl], op=ALU.mult
        )
        for s in range(hh * TK, (hh + 1) * TK):
            de_t = env["depool"].tile([128, D], F32, tag="de_t", name="de_t")
            nc.gpsimd.indirect_dma_start(
                out=de_t[:],
                out_offset=None,
                in_=de_d[:],
                in_offset=IndirectOffsetOnAxis(ap=ei32[:, s:s + 1], axis=0),
            )
            if s == 0:
                nc.vector.tensor_scalar(
                    acc[:], de_t[:], w_all[:, s:s + 1], None, op0=ALU.mult,
                )
            else:
                nc.vector.scalar_tensor_tensor(
                    out=acc[:],
                    in0=de_t[:],
                    scalar=w_all[:, s:s + 1],
                    in1=acc[:],
                    op0=ALU.mult,
                    op1=ALU.add,
                )


_NC_CACHE = {}


def _get_nc(modes=MODES):
    if modes not in _NC_CACHE:
        _NC_CACHE[modes] = build_bass(modes=modes)
    return _NC_CACHE[modes]


def kernel(hidden_states, W_up, W_down, W_q, keys, up_embed, down_embed):
    from concourse import bass2jax

    x = np.ascontiguousarray(
        np.asarray(hidden_states, dtype=np.float32).reshape(T_TOTAL, D)
    )
    shared = {
        "W_up": np.ascontiguousarray(
            np.asarray(W_up, dtype=np.float32).reshape(D // 128, 128, S)),
        "W_down": np.ascontiguousarray(np.asarray(W_down, dtype=np.float32)),
        "W_q": np.ascontiguousarray(np.asarray(W_q, dtype=np.float32)),
        "keys": np.ascontiguousarray(np.asarray(keys, dtype=np.float32)),
        "up_embed": np.ascontiguousarray(np.asarray(up_embed, dtype=np.float32)),
        "down_embed": np.ascontiguousarray(np.asarray(down_embed, dtype=np.float32)),
    }
    in_maps = [
        {"x": np.ascontiguousarray(x[c * T_CORE:(c + 1) * T_CORE]), **shared}
        for c in range(N_CORES)
    ]
    nc = _get_nc()
    res = bass2jax.run_bass_via_pjrt(nc, in_maps, n_cores=N_CORES)
    out = np.concatenate([res[c]["out"] for c in range(N_CORES)], axis=0)
    return out.reshape(1, T_TOTAL, D)
